# revision 6
# baseline (speedup 1.0000x reference)
"""Trainium2 Bass kernel for nn_BidirectionalTrustModel (histogram_binning).

Computes, per observation sequence n (N = 500000, T = 20, BINS = 12):
  1. capability edge c[n]: sequential fold over t of
       c = max(c, d)  if perf==[0,1]
       c = min(c, d)  if perf[...,0]==1
       c              otherwise
  2. trust[n] = sum_k t_k * m_k / sum_k m_k  over 12 bin centers s_k,
       m_k = (c <= s_k),  t_k = (1 + exp(beta*(dpred - s_k)))**(-zeta^2)

Key observation: trust depends on c ONLY through its bin index
b = #{k : s_k < c} (a monotone map), and monotone maps commute with the
min/max fold.  So the host recodes each (t, n) cell losslessly as a pair of
4-bit clamp params (lo, hi) in {0..11, 15} built from bucket(d) and the perf
flags — 2 bytes/cell instead of 6 — and the device scan reproduces b
EXACTLY.

Per-core layout (pure data parallel over 8 cores, no collectives):
  62500 seqs padded to 62720 = 128 partitions x 490.

Engine split per core:
  - DVE: one tensor_tensor_scan(max, min) over int8 (lo, hi) planes
    (the only intrinsically serial part, 1 elem/cycle), 12 tensor_scalar
    is_le mask ops at 4x bf16, one in-place mask*T multiply at 2x.
  - ACT (single exp/ln table, no table loads): U0 = exp(beta*dpred - beta*s_0),
    4x Ln(1 + U_k) straight out of PSUM, one big exp(mq * L), and the
    1/m = exp(-ln(12 - b)) reciprocal chain.
  - PE (idle otherwise): U_k = r^k * U0 via r^k-scaled-identity matmuls
    (bins are geometric: e^{beta(d - s_k)} = U0 * r^k, r = e^{-beta/12}),
    and the 12-bin masked-sum reduction via accumulating identity matmuls
    into PSUM — no reduction tree on DVE.
"""
import sys

if "/opt/trn_rl_repo" not in sys.path:
    sys.path.insert(0, "/opt/trn_rl_repo")

from contextlib import ExitStack

import ml_dtypes
import numpy as np

import concourse.bacc as bacc
import concourse.bass as bass
import concourse.mybir as mybir
import concourse.tile as tile
from concourse import bass_utils
from concourse.hw_specs import get_activation_tables as _orig_act_tables


def _combined_act_tables(arch):
    """Keep only natural_log_exp_and_others usable (positions preserved —
    the list index is the act_func_set_id) so Exp/Ln/Copy all resolve to ONE
    table: no ACT_TABLE_LOAD thrash between exp and ln."""
    t = _orig_act_tables(arch)
    return {k: (v if k == "natural_log_exp_and_others" else set())
            for k, v in t.items()}


bacc.get_activation_tables = _combined_act_tables

N_TOTAL = 500000
T = 20
BINS = 12
NCORES = 8
P = 128
F_CORE = 490
N_PAD = P * F_CORE  # 62720

AOT = mybir.AluOpType
ACTF = mybir.ActivationFunctionType
F32 = mybir.dt.float32
BF16 = mybir.dt.bfloat16
I8 = mybir.dt.int8

# scan tile widths (sequences per partition per tile); first smaller to
# prime the DMA->scan pipeline
FT = (35, 70, 70, 105, 105, 105)


def _steps_np():
    # bit-exact match of jnp: (arange(BINS) + 0.5) / BINS in f32
    return (np.arange(BINS, dtype=np.float32) + np.float32(0.5)) / np.float32(BINS)


def build_nc(beta: float, mq: float, ncores: int = NCORES):
    p, f = P, F_CORE
    steps = _steps_np()
    beta_f = float(np.float32(beta))
    cb0 = float(np.float32(-(np.float64(beta) * np.float64(steps[0]))))

    nc = bacc.Bacc("TRN2", target_bir_lowering=False, debug=False,
                   enable_asserts=False, num_devices=ncores)

    d_wp = nc.dram_tensor("wp", [p, 2, f, T], I8, kind="ExternalInput").ap()
    d_dpred = nc.dram_tensor("dpred", [N_PAD], F32, kind="ExternalInput").ap()
    d_wm = nc.dram_tensor("wmats", [p, 13 * 128], BF16,
                          kind="ExternalInput").ap()
    d_cb = nc.dram_tensor("cbias", [p, 2], F32, kind="ExternalInput").ap()
    d_out = nc.dram_tensor("out", [p, f], F32, kind="ExternalOutput").ap()

    with tile.TileContext(nc) as tc:
        with ExitStack() as ctx:
            inpool = ctx.enter_context(tc.tile_pool(name="in",
                                                    bufs=min(4, len(FT))))
            keep = ctx.enter_context(tc.tile_pool(name="keep", bufs=1))
            psum_u = ctx.enter_context(tc.tile_pool(name="psU", bufs=2,
                                                    space="PSUM"))
            psum_s = ctx.enter_context(tc.tile_pool(name="psS", bufs=1,
                                                    space="PSUM"))

            DP = keep.tile([p, f], F32, tag="DP")
            WT = keep.tile([p, 13 * 128], BF16, tag="WT")
            CS = keep.tile([p, f * T], BF16, tag="CS")
            U0 = keep.tile([p, f], BF16, tag="U0")
            L = keep.tile([p, BINS * f], F32, tag="L")
            Tt = keep.tile([p, BINS * f], BF16, tag="Tt")
            G = keep.tile([p, BINS * f], BF16, tag="G")
            C = keep.tile([p, f], BF16, tag="C")
            LM = keep.tile([p, f], F32, tag="LM")
            REC = keep.tile([p, f], F32, tag="REC")
            OUT = keep.tile([p, f], F32, tag="OUT")

            CBt = keep.tile([p, 2], F32, tag="CBt")

            # aux DMAs on the (otherwise idle) gpsimd queue
            nc.gpsimd.dma_start(DP[:], d_dpred.rearrange("(p n) -> p n", p=p))
            nc.gpsimd.dma_start(WT[:], d_wm)
            nc.gpsimd.dma_start(CBt[:], d_cb)

            # U0 = exp(beta*dpred - beta*s_0)  [p, f] bf16
            nc.scalar.activation(U0[:], DP[:], ACTF.Exp, bias=CBt[:, 0:1],
                                 scale=beta_f)

            # PE: U_k = r^k * U0 into PSUM (rounds of 3 banks, double
            # buffered); ACT: L_k = ln(1 + U_k) straight out of PSUM.
            for r in range(4):
                PU = psum_u.tile([p, 3 * 512], F32, tag="PU")
                for i in range(3):
                    k = 3 * r + i
                    nc.tensor.matmul(PU[:, 512 * i: 512 * i + f],
                                     WT[:, 128 * k: 128 * (k + 1)], U0[:],
                                     start=True, stop=True)
                pin = PU[:].rearrange("p (c s) -> p c s", s=512)[:, :, :f]
                lout = L[:, 3 * f * r: 3 * f * (r + 1)].rearrange(
                    "p (c n) -> p c n", c=3)
                nc.scalar.activation(lout, pin, ACTF.Ln, bias=1.0)

            # T = exp(mq * L)  [p, 12f] bf16
            nc.scalar.activation(Tt[:], L[:], ACTF.Exp,
                                 scale=float(np.float32(mq)))

            # phase A: pipelined DMA + scan over (lo, hi) int8 planes.
            # slot-0 carries lo=hi=v0 so state = v0 exactly regardless of
            # the carry-in: sequences pack back-to-back in ONE flat stream.
            base = 0
            for ftj in FT:
                FT20 = ftj * T
                WPt = inpool.tile([p, 2 * FT20], I8, tag="WPt")
                nc.sync.dma_start(
                    WPt[:].rearrange("p (c n t) -> p c n t", c=2, t=T),
                    d_wp[:, :, base:base + ftj, :])
                nc.vector.tensor_tensor_scan(CS[:, T * base: T * (base + ftj)],
                                             WPt[:, 0:FT20], WPt[:, FT20:],
                                             0.0, AOT.max, AOT.min)
                base += ftj

            # b per sequence = scan state at t = T-1
            cs_v = CS[:].rearrange("p (n t) -> p n t", t=T)[:, :, T - 1]
            nc.vector.tensor_copy(C[:], cs_v)

            # G_k = (b <= k), 4x-mode tensor_scalar ops
            for k in range(BINS):
                nc.vector.tensor_scalar(G[:, f * k: f * (k + 1)], C[:],
                                        float(k), None, AOT.is_le)
            # masked T in place: G *= T  (2x tensor_tensor)
            nc.vector.tensor_tensor(G[:], G[:], Tt[:], AOT.mult)

            # 1/m = exp(-ln(12 - b)) on ACT (exact to f32 table accuracy)
            nc.scalar.activation(LM[:], C[:], ACTF.Ln, bias=CBt[:, 1:2],
                                 scale=-1.0)
            nc.scalar.activation(REC[:], LM[:], ACTF.Exp, scale=-1.0)

            # PE: tsum = sum_k G_k via accumulating identity matmuls
            TS = psum_s.tile([p, 512], F32, tag="TS")
            for k in range(BINS):
                nc.tensor.matmul(TS[:, :f], WT[:, 12 * 128: 13 * 128],
                                 G[:, f * k: f * (k + 1)],
                                 start=(k == 0), stop=(k == BINS - 1))

            # trust = tsum * (1/m)
            nc.vector.tensor_tensor(OUT[:], TS[:, :f], REC[:], AOT.mult)
            nc.sync.dma_start(d_out, OUT[:])

    nc.compile()
    return nc


_CACHE: dict = {}


def _get_nc(beta: float, mq: float):
    key = (beta, mq)
    if key not in _CACHE:
        _CACHE[key] = build_nc(beta, mq)
    return _CACHE[key]


def make_in_maps(inptasksperf, difficulties_obs, difficulties_pred,
                 n_total=N_TOTAL, ncores=NCORES, n_pad=N_PAD, p=P):
    """Host-side recode: bucket(d_obs) + perf flags -> (lo, hi) int8 clamp
    params, shard + pad + t-inner relayout."""
    perf = np.asarray(inptasksperf)
    dobs = np.asarray(difficulties_obs, dtype=np.float32)[..., 0]    # [T, N]
    dpred = np.asarray(difficulties_pred, dtype=np.float32)[..., 0]  # [N]
    f = n_pad // p
    nc_n = n_total // ncores
    steps = _steps_np()

    # b = #{k : s_k < d} in 0..11 (exact f32 comparisons, matches the
    # reference's mask since bucketing commutes with the min/max fold)
    b = np.searchsorted(steps, dobs.ravel(), side="left").astype(
        np.int8).reshape(dobs.shape)
    p0 = perf[..., 0] != 0
    p1 = perf[..., 1] != 0
    suc = p1 & ~p0
    lo = np.where(suc, b, 0).astype(np.int8)
    hi = np.where(p0, b, np.int8(15)).astype(np.int8)
    # slot-0 self-reset: state after step 0 is exactly v0
    v0 = np.where(suc[0], b[0], 0).astype(np.int8)
    lo[0] = v0
    hi[0] = v0

    in_maps = []
    for c in range(ncores):
        sl = slice(c * nc_n, (c + 1) * nc_n)
        lop = np.zeros((T, n_pad), np.int8)
        lop[:, :nc_n] = lo[:, sl]
        hip = np.zeros((T, n_pad), np.int8)
        hip[:, :nc_n] = hi[:, sl]
        loc = lop.reshape(T, p, f).transpose(1, 2, 0)   # [p, f, T]
        hic = hip.reshape(T, p, f).transpose(1, 2, 0)
        wp = np.ascontiguousarray(np.stack([loc, hic], axis=1))  # [p,2,f,T]

        dpc = np.zeros((n_pad,), np.float32)
        dpc[:nc_n] = dpred[sl]
        in_maps.append({"wp": wp, "dpred": dpc})
    return in_maps


def make_consts(beta, p=P):
    """Per-bin scaled identities r^k * I (bf16) + plain identity, packed as
    [p, 13*128] for the PE stationary slices."""
    r = np.exp(-np.float64(beta) * np.arange(13) / 12.0)  # r^k, r^12 unused
    W = np.zeros((13, p, 128), np.float32)
    eye = np.eye(p, 128, dtype=np.float32)
    for k in range(12):
        W[k] = eye * np.float32(r[k])
    W[12] = eye
    wm = W.transpose(1, 0, 2).reshape(p, 13 * 128)
    steps = _steps_np()
    cb0 = np.float32(-(np.float64(beta) * np.float64(steps[0])))
    cb = np.broadcast_to(np.array([cb0, np.float32(BINS)], np.float32),
                         (p, 2))
    return {"wmats": wm.astype(ml_dtypes.bfloat16),
            "cbias": np.ascontiguousarray(cb)}


def kernel(inptasksobs=None, inptasksperf=None, inptaskspred=None,
           num_obs_tasks=None, tasksobsids=None, taskspredids=None,
           difficulties_obs=None, difficulties_pred=None,
           betas=None, zetas=None, **_):
    beta = float(np.float32(np.asarray(betas).reshape(-1)[0]))
    zeta = np.float32(np.asarray(zetas).reshape(-1)[0])
    mq = float(np.float32(-(zeta * zeta)))

    nc = _get_nc(beta, mq)
    in_maps = make_in_maps(inptasksperf, difficulties_obs, difficulties_pred)
    consts = make_consts(beta)
    for m in in_maps:
        m.update(consts)
    res = bass_utils.run_bass_kernel_spmd(nc, in_maps,
                                          core_ids=list(range(NCORES)))
    nc_n = N_TOTAL // NCORES
    parts = [np.asarray(r["out"]).reshape(-1)[:nc_n] for r in res.results]
    return np.concatenate(parts).reshape(N_TOTAL, 1).astype(np.float32)


if __name__ == "__main__":
    rng = np.random.default_rng(0)
    ins = {
        "inptasksperf": rng.integers(0, 2, (T, N_TOTAL, 2)).astype(np.int32),
        "difficulties_obs": (0.9 * rng.random((T, N_TOTAL, 1))).astype(np.float32),
        "difficulties_pred": (0.9 * rng.random((N_TOTAL, 1))).astype(np.float32),
        "betas": np.array([7.0], np.float32),
        "zetas": np.array([0.5], np.float32),
    }
    out = kernel(**ins)
    print(out.shape, out.dtype, out[:5, 0])


# revision 8
# speedup vs baseline: 1.1300x; 1.1300x over previous
"""Trainium2 Bass kernel for nn_BidirectionalTrustModel (histogram_binning).

Computes, per observation sequence n (N = 500000, T = 20, BINS = 12):
  1. capability edge c[n]: sequential fold over t of
       c = max(c, d)  if perf==[0,1]
       c = min(c, d)  if perf[...,0]==1
       c              otherwise
  2. trust[n] = sum_k t_k * m_k / sum_k m_k  over 12 bin centers s_k,
       m_k = (c <= s_k),  t_k = (1 + exp(beta*(dpred - s_k)))**(-zeta^2)

Key observation: trust depends on c ONLY through its bin index
b = #{k : s_k < c} (a monotone map), and monotone maps commute with the
min/max fold.  So the host recodes each (t, n) cell losslessly as a pair of
4-bit clamp params (lo, hi) in {0..11, 15} built from bucket(d) and the perf
flags — 2 bytes/cell instead of 6 — and the device scan reproduces b
EXACTLY.

Per-core layout (pure data parallel over 8 cores, no collectives):
  62500 seqs padded to 62720 = 128 partitions x 490.

Engine split per core:
  - DVE: one tensor_tensor_scan(max, min) over int8 (lo, hi) planes
    (intrinsically serial, ~2 cyc/elem), 12 tensor_scalar is_le mask ops at
    4x bf16, chunked in-place mask*T multiplies at 2x.
  - ACT (single exp/ln table): U0 = exp(beta*dpred - beta*s_0), 4x
    Ln(1 + U_k) straight out of PSUM, one big exp(mq * L), and the
    1/m = exp(-ln(12 - b)) reciprocal chain (scheduled late via wait hint
    so it cannot stall the big exp behind the scan chain).
  - PE (idle otherwise): U_k = r^k * U0 via r^k-scaled-identity matmuls
    (bins are geometric: e^{beta(d - s_k)} = U0 * r^k, r = e^{-beta/12}),
    and the 12-bin masked-sum via accumulating identity matmuls into PSUM,
    interleaved with the DVE mask-mult chunks.
"""
import sys

if "/opt/trn_rl_repo" not in sys.path:
    sys.path.insert(0, "/opt/trn_rl_repo")

from contextlib import ExitStack

import ml_dtypes
import numpy as np

import concourse.bacc as bacc
import concourse.bass as bass
import concourse.mybir as mybir
import concourse.tile as tile
from concourse import bass_utils
from concourse.hw_specs import get_activation_tables as _orig_act_tables


def _combined_act_tables(arch):
    """Keep only natural_log_exp_and_others usable (positions preserved —
    the list index is the act_func_set_id) so Exp/Ln/Copy all resolve to ONE
    table: no ACT_TABLE_LOAD thrash between exp and ln."""
    t = _orig_act_tables(arch)
    return {k: (v if k == "natural_log_exp_and_others" else set())
            for k, v in t.items()}


bacc.get_activation_tables = _combined_act_tables

N_TOTAL = 500000
T = 20
BINS = 12
NCORES = 8
P = 128
F_CORE = 490
N_PAD = P * F_CORE  # 62720

AOT = mybir.AluOpType
ACTF = mybir.ActivationFunctionType
F32 = mybir.dt.float32
BF16 = mybir.dt.bfloat16
I8 = mybir.dt.int8

# scan tile widths (sequences per partition per tile); first smaller to
# prime the DMA->scan pipeline
FT = (20, 50, 100, 110, 105, 105)
# mask-mult chunking (bins per chunk) to interleave DVE mult / PE accumulate
GT_CHUNK = 3


def _steps_np():
    # bit-exact match of jnp: (arange(BINS) + 0.5) / BINS in f32
    return (np.arange(BINS, dtype=np.float32) + np.float32(0.5)) / np.float32(BINS)


def build_nc(beta: float, mq: float, ncores: int = NCORES):
    p, f = P, F_CORE
    beta_f = float(np.float32(beta))

    nc = bacc.Bacc("TRN2", target_bir_lowering=False, debug=False,
                   enable_asserts=False, num_devices=ncores)

    d_wp = nc.dram_tensor("wp", [p, 2, f, T], I8, kind="ExternalInput").ap()
    d_dpred = nc.dram_tensor("dpred", [N_PAD], F32, kind="ExternalInput").ap()
    d_wm = nc.dram_tensor("wmats", [p, 13 * 128], BF16,
                          kind="ExternalInput").ap()
    d_cb = nc.dram_tensor("cbias", [p, 2], F32, kind="ExternalInput").ap()
    d_out = nc.dram_tensor("out", [p, f], F32, kind="ExternalOutput").ap()

    with tile.TileContext(nc) as tc:
        with ExitStack() as ctx:
            inpool = ctx.enter_context(tc.tile_pool(name="in",
                                                    bufs=min(4, len(FT))))
            keep = ctx.enter_context(tc.tile_pool(name="keep", bufs=1))
            psum_u = ctx.enter_context(tc.tile_pool(name="psU", bufs=2,
                                                    space="PSUM"))
            psum_s = ctx.enter_context(tc.tile_pool(name="psS", bufs=1,
                                                    space="PSUM"))

            DP = keep.tile([p, f], F32, tag="DP")
            WT = keep.tile([p, 13 * 128], BF16, tag="WT")
            CS = keep.tile([p, f * T], BF16, tag="CS")
            U0 = keep.tile([p, f], BF16, tag="U0")
            L = keep.tile([p, BINS * f], F32, tag="L")
            Tt = keep.tile([p, BINS * f], BF16, tag="Tt")
            G = keep.tile([p, BINS * f], BF16, tag="G")
            C = keep.tile([p, f], BF16, tag="C")
            LM = keep.tile([p, f], F32, tag="LM")
            REC = keep.tile([p, f], F32, tag="REC")
            OUT = keep.tile([p, f], F32, tag="OUT")
            CBt = keep.tile([p, 2], F32, tag="CBt")
            DUM = keep.tile([p, 1], F32, tag="DUM")

            # warm the ACT table during the preamble window: memset + tiny
            # exp forces the one ACT_TABLE_LOAD before any real dependency
            nc.gpsimd.memset(DUM[:], 0.0)
            nc.scalar.activation(DUM[:], DUM[:], ACTF.Exp)

            # aux DMAs: small ones (needed early) on gpsimd queue, the
            # 416KB weight pack on the tensor queue (PE consumes it)
            nc.gpsimd.dma_start(CBt[:], d_cb)
            nc.gpsimd.dma_start(DP[:], d_dpred.rearrange("(p n) -> p n", p=p))
            nc.scalar.dma_start(WT[:], d_wm)

            # U0 = exp(beta*dpred - beta*s_0)  [p, f] bf16
            nc.scalar.activation(U0[:], DP[:], ACTF.Exp, bias=CBt[:, 0:1],
                                 scale=beta_f)

            # PE: U_k = r^k * U0 into PSUM (rounds of 3 banks, double
            # buffered); ACT: L_k = ln(1 + U_k) straight out of PSUM.
            for r in range(4):
                PU = psum_u.tile([p, 3 * 512], F32, tag="PU")
                for i in range(3):
                    k = 3 * r + i
                    nc.tensor.matmul(PU[:, 512 * i: 512 * i + f],
                                     WT[:, 128 * k: 128 * (k + 1)], U0[:],
                                     start=True, stop=True)
                pin = PU[:].rearrange("p (c s) -> p c s", s=512)[:, :, :f]
                lout = L[:, 3 * f * r: 3 * f * (r + 1)].rearrange(
                    "p (c n) -> p c n", c=3)
                nc.scalar.activation(lout, pin, ACTF.Ln, bias=1.0)

            # T = exp(mq * L)  [p, 12f] bf16
            nc.scalar.activation(Tt[:], L[:], ACTF.Exp,
                                 scale=float(np.float32(mq)))

            # phase A: pipelined DMA + scan over (lo, hi) int8 planes.
            # slot-0 carries lo=hi=v0 so state = v0 exactly regardless of
            # the carry-in: sequences pack back-to-back in ONE flat stream.
            base = 0
            for ftj in FT:
                FT20 = ftj * T
                WPt = inpool.tile([p, 2 * FT20], I8, tag="WPt")
                nc.sync.dma_start(
                    WPt[:].rearrange("p (c n t) -> p c n t", c=2, t=T),
                    d_wp[:, :, base:base + ftj, :])
                nc.vector.tensor_tensor_scan(CS[:, T * base: T * (base + ftj)],
                                             WPt[:, 0:FT20], WPt[:, FT20:],
                                             0.0, AOT.max, AOT.min)
                base += ftj

            # b per sequence = scan state at t = T-1
            cs_v = CS[:].rearrange("p (n t) -> p n t", t=T)[:, :, T - 1]
            nc.vector.tensor_copy(C[:], cs_v)

            # G_k = (b <= k), 4x-mode tensor_scalar ops
            for k in range(BINS):
                nc.vector.tensor_scalar(G[:, f * k: f * (k + 1)], C[:],
                                        float(k), None, AOT.is_le)

            # 1/m = exp(-ln(12 - b)) on ACT — wait-hinted late so the
            # static ACT queue order keeps it AFTER the big exp above
            with tc.tile_wait_until(0.020):
                nc.scalar.activation(LM[:], C[:], ACTF.Ln, bias=CBt[:, 1:2],
                                     scale=-1.0)
                nc.scalar.activation(REC[:], LM[:], ACTF.Exp, scale=-1.0)

            # masked T chunks on DVE interleaved with PE PSUM accumulation
            TS = psum_s.tile([p, 512], F32, tag="TS")
            n_chunks = BINS // GT_CHUNK
            for c in range(n_chunks):
                k0 = c * GT_CHUNK
                sl = slice(f * k0, f * (k0 + GT_CHUNK))
                nc.vector.tensor_tensor(G[:, sl], G[:, sl], Tt[:, sl],
                                        AOT.mult)
                with tc.tile_wait_until(0.020 + 0.001 * c):
                    for i in range(GT_CHUNK):
                        k = k0 + i
                        nc.tensor.matmul(TS[:, :f],
                                         WT[:, 12 * 128: 13 * 128],
                                         G[:, f * k: f * (k + 1)],
                                         start=(k == 0),
                                         stop=(k == BINS - 1))

            # trust = tsum * (1/m)
            nc.vector.tensor_tensor(OUT[:], TS[:, :f], REC[:], AOT.mult)
            nc.sync.dma_start(d_out, OUT[:])

    nc.compile()
    return nc


_CACHE: dict = {}


def _get_nc(beta: float, mq: float):
    key = (beta, mq)
    if key not in _CACHE:
        _CACHE[key] = build_nc(beta, mq)
    return _CACHE[key]


def make_in_maps(inptasksperf, difficulties_obs, difficulties_pred,
                 n_total=N_TOTAL, ncores=NCORES, n_pad=N_PAD, p=P):
    """Host-side recode: bucket(d_obs) + perf flags -> (lo, hi) int8 clamp
    params, shard + pad + t-inner relayout."""
    perf = np.asarray(inptasksperf)
    dobs = np.asarray(difficulties_obs, dtype=np.float32)[..., 0]    # [T, N]
    dpred = np.asarray(difficulties_pred, dtype=np.float32)[..., 0]  # [N]
    f = n_pad // p
    nc_n = n_total // ncores
    steps = _steps_np()

    # b = #{k : s_k < d} in 0..11 (exact f32 comparisons, matches the
    # reference's mask since bucketing commutes with the min/max fold)
    b = np.searchsorted(steps, dobs.ravel(), side="left").astype(
        np.int8).reshape(dobs.shape)
    p0 = perf[..., 0] != 0
    p1 = perf[..., 1] != 0
    suc = p1 & ~p0
    lo = np.where(suc, b, 0).astype(np.int8)
    hi = np.where(p0, b, np.int8(15)).astype(np.int8)
    # slot-0 self-reset: state after step 0 is exactly v0
    v0 = np.where(suc[0], b[0], 0).astype(np.int8)
    lo[0] = v0
    hi[0] = v0

    in_maps = []
    for c in range(ncores):
        sl = slice(c * nc_n, (c + 1) * nc_n)
        lop = np.zeros((T, n_pad), np.int8)
        lop[:, :nc_n] = lo[:, sl]
        hip = np.zeros((T, n_pad), np.int8)
        hip[:, :nc_n] = hi[:, sl]
        loc = lop.reshape(T, p, f).transpose(1, 2, 0)   # [p, f, T]
        hic = hip.reshape(T, p, f).transpose(1, 2, 0)
        wp = np.ascontiguousarray(np.stack([loc, hic], axis=1))  # [p,2,f,T]

        dpc = np.zeros((n_pad,), np.float32)
        dpc[:nc_n] = dpred[sl]
        in_maps.append({"wp": wp, "dpred": dpc})
    return in_maps


def make_consts(beta, p=P):
    """Per-bin scaled identities r^k * I (bf16) + plain identity, packed as
    [p, 13*128] for the PE stationary slices, plus activation bias consts."""
    r = np.exp(-np.float64(beta) * np.arange(13) / 12.0)  # r^k, r^12 unused
    W = np.zeros((13, p, 128), np.float32)
    eye = np.eye(p, 128, dtype=np.float32)
    for k in range(12):
        W[k] = eye * np.float32(r[k])
    W[12] = eye
    wm = W.transpose(1, 0, 2).reshape(p, 13 * 128)
    steps = _steps_np()
    cb0 = np.float32(-(np.float64(beta) * np.float64(steps[0])))
    cb = np.broadcast_to(np.array([cb0, np.float32(BINS)], np.float32),
                         (p, 2))
    return {"wmats": wm.astype(ml_dtypes.bfloat16),
            "cbias": np.ascontiguousarray(cb)}


def kernel(inptasksobs=None, inptasksperf=None, inptaskspred=None,
           num_obs_tasks=None, tasksobsids=None, taskspredids=None,
           difficulties_obs=None, difficulties_pred=None,
           betas=None, zetas=None, **_):
    beta = float(np.float32(np.asarray(betas).reshape(-1)[0]))
    zeta = np.float32(np.asarray(zetas).reshape(-1)[0])
    mq = float(np.float32(-(zeta * zeta)))

    nc = _get_nc(beta, mq)
    in_maps = make_in_maps(inptasksperf, difficulties_obs, difficulties_pred)
    consts = make_consts(beta)
    for m in in_maps:
        m.update(consts)
    res = bass_utils.run_bass_kernel_spmd(nc, in_maps,
                                          core_ids=list(range(NCORES)))
    nc_n = N_TOTAL // NCORES
    parts = [np.asarray(r["out"]).reshape(-1)[:nc_n] for r in res.results]
    return np.concatenate(parts).reshape(N_TOTAL, 1).astype(np.float32)


if __name__ == "__main__":
    rng = np.random.default_rng(0)
    ins = {
        "inptasksperf": rng.integers(0, 2, (T, N_TOTAL, 2)).astype(np.int32),
        "difficulties_obs": (0.9 * rng.random((T, N_TOTAL, 1))).astype(np.float32),
        "difficulties_pred": (0.9 * rng.random((N_TOTAL, 1))).astype(np.float32),
        "betas": np.array([7.0], np.float32),
        "zetas": np.array([0.5], np.float32),
    }
    out = kernel(**ins)
    print(out.shape, out.dtype, out[:5, 0])
